# revision 42
# baseline (speedup 1.0000x reference)
"""HSTU layer kernel for Trainium2, 8 NeuronCores.

Sharding: core = 4*b + hg  (b in {0,1} data-parallel over batch,
hg in {0..3} head-parallel: 4 heads = 256 channels of U/V/Q/K each).

v23 @ ~88us (v10 baseline 105us, stub 220us). Key techniques on top of
v10's bf16/Toeplitz-mask/host-LN-combine design:
  - QK logits: row-tiled CONCURRENT head pairs -- two K=64 matmuls at
    tile_position (0,0)/(64,0) writing two psum banks run simultaneously
    (HW-verified: the 2nd matmul of a pair retires in ~4ns), so a head
    pair costs one 512-col slot instead of two.
  - AV: col-tiled CONCURRENT head pairs -- two M=64 matmuls writing
    partitions 0-63 / 64-127 of the SAME psum bank, accumulated over kt
    (per-head V layout [t%128, kt, head, 64]; no zero-padding needed).
    AV pairs flush at depth 1 behind the SILU (tighter = faster).
  - rab floor: for rt1 kt<=2 every key-query distance is >=128, where
    the T5 log-bucket saturates at bucket 31 -> the mask is a per-head
    CONSTANT; those (widest) tiles skip the DVE mask pass entirely and
    run silu(alpha*logits + c_h) on ACT straight from psum.
  - in_proj wave 0 computes Q AND U with 8 live psum groups, dc-outer,
    so the PE keeps pace with the input DMA stream; K re-reads resident
    chunks densely. V runs as forced fill units inside the attention
    loops (attention starts right after K).
  - ~4us of dummy matmuls warm the HAM clock gate during the initial
    DMA wait and bridge the final invd->nUA->A dependency chain
    (PE idle >3.4us re-throttles the clock to 1.2GHz).
  - attention processes rt1 before rt0 so the tail chain is the short,
    causal-trimmed one; B/A out_proj groups + stats pop from a work
    queue between attention batches; psum->sbuf copies alternate
    2:1 DVE/ACT; gU/nUA/sqh on DVE/GpSimd (GpSimd cannot touch PSUM,
    and its tensor_scalar op pays a ~15us one-time library load).

Per core (channels-on-partitions, zero device transposes):
  qk^T = silu(W_qk @ x^T + b)            [128, 4 ct, t] bf16
  U    = silu(W_u @ x^T + b)             [128, 2 g, t] bf16; gU = gamma*U
  V    = silu(x @ W_v^T + b_v) * vmask   [128, 8 kt, 4 head, 64] bf16
  logits^T[j,i] = K_h @ Q_h^T            (psum f32, row-tiled pairs)
  att^T = silu(alpha*logits^T + MT_win)  bf16   (MT Toeplitz window table)
  AVs^T = (V_h^T @ att^T) * invd         [128, 2 g, t] f32r (col-tiled pairs)
  s1 = sum_c AVs, s2 = sum_c AVs^2       (ones-matmul, contraction 128)
  A^T = WO^T @ (AVs*gU)^T ; B^T = WO^T @ gU^T   (out_proj partials)

Host combine (LayerNorm is linear in its input given row stats):
  y = rho*A - (rho*mu)*B (+ C) + b_out + x
"""
import math
import numpy as np
import ml_dtypes

import concourse.bass as bass
import concourse.mybir as mybir
import concourse.tile as tile
from concourse import bacc
from concourse.bass_utils import run_bass_kernel_spmd

NUM_HEADS = 16
NUM_BUCKETS = 32
MAX_DISTANCE = 128
NEG_INF = -1e9
LN_EPS = 1e-5

B, T, D = 2, 1024, 1024
H_PER = 4           # heads per core
F32R = mybir.dt.float32r
F32 = mybir.dt.float32
BF16 = mybir.dt.bfloat16
BF16_NP = ml_dtypes.bfloat16

# packed-small-input column offsets: PKA = biases/gamma/beta/vmask/bvb
# (tiny, lands immediately so the first SILUs never wait), PKB = invd
PK_BQK, PK_BU, PK_GAM, PK_BET = 0, 4, 6, 8
PK_VM, PK_BVB, PK_RABF = 10, 18, 274
PKA_N = 278
PKB_N = 1024

LAST_RESULTS = None


def _bucket_np(n):
    """T5-style log bucket for clamped distance n >= 0."""
    max_exact = NUM_BUCKETS // 2
    with np.errstate(divide="ignore", invalid="ignore"):
        large = max_exact + (
            np.log(n.astype(np.float32) / max_exact + 1e-6)
            / math.log(MAX_DISTANCE / max_exact)
            * (NUM_BUCKETS - max_exact)
        ).astype(np.int32)
    large = np.minimum(large, NUM_BUCKETS - 1)
    return np.where(n < max_exact, n, large)


def _toeplitz_mask_np(rab_emb, heads):
    """MT[p, jj, c] = rab_h(c - p) for c-p >= 0 else -1e9.  [128, 4, 1024]"""
    d = np.arange(T)[None, :] - np.arange(128)[:, None]   # [128, 1024] = c - p
    n = np.clip(d, 0, None)
    buckets = _bucket_np(n)                               # [128, 1024]
    out = np.empty((128, len(heads), T), np.float32)
    for jj, h in enumerate(heads):
        out[:, jj, :] = np.where(d < 0, NEG_INF, rab_emb[buckets, h])
    return np.ascontiguousarray(out)


def _build(with_c):
    nc = bacc.Bacc("TRN2", target_bir_lowering=False, debug=False, num_devices=8)

    def inp(name, shape, dt):
        return nc.dram_tensor(name, shape, dt, kind="ExternalInput").ap()

    XC = inp("XC", [8, 128, 1024], BF16)     # x[b].T d-chunks
    WC = inp("WC", [8, 128, 768], BF16)      # W_in.T d-chunks: [U(256)|Q(256)|K(256)]
    WVC = inp("WVC", [2, 128, 4, 256], BF16)  # W_v.T d-chunks, 2 bundles
    WO = inp("WO", [128, 2, 1024], BF16)     # W_out cols slice: [c%128, c//128, dout]
    MT = inp("MT", [128, 4, 1024], BF16)     # Toeplitz mask windows per local head
    PKA = inp("PKA", [128, PKA_N], F32)      # biases/gamma/beta/vmask/bvb
    PKB = inp("PKB", [128, PKB_N], F32)      # invd (per-query 1/denom rows)
    ONESP = inp("ONESP", [128, 1], F32R)

    AT = nc.dram_tensor("AT", [8, 128, 1024], BF16, kind="ExternalOutput").ap()
    BT = nc.dram_tensor("BT", [8, 128, 1024], BF16, kind="ExternalOutput").ap()
    CT = (nc.dram_tensor("CT", [8, 128, 1024], BF16, kind="ExternalOutput").ap()
          if with_c else None)
    SOUT = nc.dram_tensor("SOUT", [1, 4, 512], F32, kind="ExternalOutput").ap()

    alpha = (D // NUM_HEADS) ** (-0.5)
    SILU = mybir.ActivationFunctionType.Silu
    IDENT = mybir.ActivationFunctionType.Identity
    MULT = mybir.AluOpType.mult
    ADD = mybir.AluOpType.add

    with tile.TileContext(nc) as tc:
        with (
            tc.tile_pool(name="big", bufs=1) as big,
            tc.tile_pool(name="scratch", bufs=1) as scratch,
            tc.tile_pool(name="apool", bufs=8) as apool,
            tc.tile_pool(name="opool", bufs=8) as opool,
            tc.tile_pool(name="ps", bufs=3, space="PSUM") as ps,
            tc.tile_pool(name="psqk", bufs=2, space="PSUM") as psqk,
            tc.tile_pool(name="psav", bufs=1, space="PSUM") as psav,
        ):
            # ---- resident loads: x chunks on sync, W chunks on scalar so
            # issue serialization doesn't gate the in_proj start ----
            PKs = big.tile([128, PKA_N], F32)
            nc.sync.dma_start(PKs[:], PKA[:])
            xcs, wcs = [], []
            for dc in range(8):
                xt = big.tile([128, 1024], BF16, tag=f"xc{dc}")
                nc.sync.dma_start(xt[:], XC[dc])
                xcs.append(xt)
            for dc in range(8):
                wt = big.tile([128, 768], BF16, tag=f"wc{dc}")
                nc.scalar.dma_start(wt[:], WC[dc])
                wcs.append(wt)
            wv2 = []
            for bi in range(2):
                wv = big.tile([128, 4, 256], BF16, tag=f"wv{bi}")
                (nc.sync if bi == 0 else nc.scalar).dma_start(wv[:], WVC[bi])
                wv2.append(wv)
            WOs = big.tile([128, 2, 1024], BF16)
            nc.scalar.dma_start(WOs[:], WO[:])
            MTs = big.tile([128, 4, 1024], BF16)
            nc.sync.dma_start(MTs[:], MT[:])
            PKi = big.tile([128, PKB_N], F32)
            nc.sync.dma_start(PKi[:], PKB[:])
            ONESs = big.tile([128, 1], F32R)
            nc.scalar.dma_start(ONESs[:], ONESP[:])

            # ---- HAM warmup: ~3.5us of dummy matmuls on a memset tile
            # during the initial DMA wait flips the PE clock gate to 8/8
            # before the first real matmul ----
            wrm = big.tile([128, 512], BF16, tag="wrm")
            nc.vector.memset(wrm[:], 0.0)
            wps = ps.tile([128, 512], F32, tag="mm", name="warm")
            for i in range(10):
                nc.tensor.matmul(wps[:], wrm[:, 0:128], wrm[:],
                                 start=True, stop=True,
                                 skip_group_check=True)

            # ---- in_proj Q+U then K: dc-OUTER so each x/W chunk is
            # consumed as soon as its DMA lands. Wave 0 runs 8 live psum
            # groups (Q ct0,1 x th + U g x th) across all 8 banks so the
            # PE keeps pace with the input stream; wave 1 (K) re-reads
            # resident chunks densely. ----
            qk = big.tile([128, 4, 1024], BF16)  # ct 0,1=Q  2,3=K
            U128 = big.tile([128, 2, 1024], BF16)
            gU = scratch.tile([128, 2, 1024], BF16, tag="gu")
            bU = (scratch.tile([128, 2, 1024], BF16, tag="bu")
                  if with_c else None)

            pdq = psqk.tile([128, 2, 512], F32, tag="qk", name="pdq")
            pdu = psqk.tile([128, 2, 512], F32, tag="qk", name="pdu")
            q_pts = [ps.tile([128, 512], F32, tag="mm", name=f"ptq{i}")
                     for i in range(2)] + [pdq[:, 0, :], pdq[:, 1, :]]
            u_pts = [ps.tile([128, 512], F32, tag="mm", name="ptu0"),
                     psav.tile([128, 512], F32, tag="av", name="ptu1"),
                     pdu[:, 0, :], pdu[:, 1, :]]
            q_groups = [(c, t) for c in range(2) for t in range(2)]
            u_groups = [(g, t) for g in range(2) for t in range(2)]
            for dc in range(8):
                for gi, (ct, th) in enumerate(q_groups):
                    nc.tensor.matmul(
                        q_pts[gi][:],
                        wcs[dc][:, 256 + ct * 128:256 + (ct + 1) * 128],
                        xcs[dc][:, th * 512:(th + 1) * 512],
                        start=(dc == 0), stop=(dc == 7),
                        skip_group_check=True,
                    )
                for gi, (g, th) in enumerate(u_groups):
                    nc.tensor.matmul(
                        u_pts[gi][:],
                        wcs[dc][:, g * 128:(g + 1) * 128],
                        xcs[dc][:, th * 512:(th + 1) * 512],
                        start=(dc == 0), stop=(dc == 7),
                        skip_group_check=True,
                    )
            for gi in range(2):
                nc.scalar.activation(
                    qk[:, 0, gi * 512:(gi + 1) * 512], q_pts[gi][:],
                    SILU, bias=PKs[:, PK_BQK:PK_BQK + 1], scale=1.0,
                )
            nc.scalar.activation(
                qk[:, 1, :], pdq[:, :, :],
                SILU, bias=PKs[:, PK_BQK + 1:PK_BQK + 2], scale=1.0,
            )
            for gi in range(2):
                nc.scalar.activation(
                    U128[:, 0, gi * 512:(gi + 1) * 512], u_pts[gi][:],
                    SILU, bias=PKs[:, PK_BU:PK_BU + 1], scale=1.0,
                )
            nc.scalar.activation(
                U128[:, 1, :], pdu[:, :, :],
                SILU, bias=PKs[:, PK_BU + 1:PK_BU + 2], scale=1.0,
            )
            for g in range(2):
                nc.vector.tensor_scalar_mul(
                    gU[:, g, :], U128[:, g, :],
                    PKs[:, PK_GAM + g:PK_GAM + g + 1])
                if with_c:
                    nc.vector.tensor_scalar_mul(
                        bU[:, g, :], U128[:, g, :],
                        PKs[:, PK_BET + g:PK_BET + g + 1])

            # wave 1: K (ct 2,3), resident chunks, dense
            pdk = psqk.tile([128, 2, 512], F32, tag="qk", name="pdk")
            k_pts = [ps.tile([128, 512], F32, tag="mm", name=f"ptk{i}")
                     for i in range(2)] + [pdk[:, 0, :], pdk[:, 1, :]]
            for dc in range(8):
                for gi, (ct, th) in enumerate(
                        [(2 + c, t) for c in range(2) for t in range(2)]):
                    nc.tensor.matmul(
                        k_pts[gi][:],
                        wcs[dc][:, 256 + ct * 128:256 + (ct + 1) * 128],
                        xcs[dc][:, th * 512:(th + 1) * 512],
                        start=(dc == 0), stop=(dc == 7),
                        skip_group_check=True,
                    )
            for gi in range(2):
                nc.scalar.activation(
                    qk[:, 2, gi * 512:(gi + 1) * 512], k_pts[gi][:],
                    SILU, bias=PKs[:, PK_BQK + 2:PK_BQK + 3], scale=1.0,
                )
            nc.scalar.activation(
                qk[:, 3, :], pdk[:, :, :],
                SILU, bias=PKs[:, PK_BQK + 3:PK_BQK + 4], scale=1.0,
            )

            # ---- in_proj natural: V = silu(x @ W_V^T + b_V) * vmask ----
            # V4 [t%128, kt, head, 64]: per-head natural layout for the
            # col-tiled AV pairs (head channels at psum partitions
            # (j%2)*64 via tile_position). Emitted as fill units inside
            # the attention loops; v_emit(tt) force-emits units so every
            # AV flush for kt has its V4 writer emitted first (Tile
            # dependencies follow program order).
            V4 = big.tile([128, 8, 4, 64], BF16)
            v_next = [0]

            def v_unit(tt):
                pt = ps.tile([128, 512], F32, tag="mm", name=f"vpt{tt}")
                for dc in range(8):
                    nc.tensor.matmul(
                        pt[:, :256],
                        xcs[dc][:, tt * 128:(tt + 1) * 128],
                        wv2[dc // 4][:, dc % 4, :],
                        start=(dc == 0), stop=(dc == 7),
                    )
                vs = apool.tile([128, 256], F32, tag="vs")
                nc.vector.tensor_add(vs[:], pt[:, :256],
                                     PKs[:, PK_BVB:PK_BVB + 256])
                nc.scalar.activation(
                    V4[:, tt, :, :], vs[:], SILU,
                    scale=PKs[:, PK_VM + tt:PK_VM + tt + 1])

            def v_emit(upto):
                while v_next[0] <= upto:
                    v_unit(v_next[0])
                    v_next[0] += 1

            # ---- fill-work units, popped inside the attention loops so the
            # PE always has independent matmuls queued ----
            copy_flip = [0]

            def pcopy(dst, src):
                """psum -> sbuf bf16 copy, 2:1 DVE / ACT."""
                copy_flip[0] = (copy_flip[0] + 1) % 3
                if copy_flip[0]:
                    nc.vector.tensor_copy(out=dst, in_=src)
                else:
                    nc.scalar.activation(dst, src, IDENT)

            def make_bgroup(dst, rhs, dt_, th, nm):
                def emit():
                    pt = ps.tile([128, 512], F32, tag="mm",
                                 name=f"bpt_{nm}_{dt_}_{th}")
                    for g in range(2):
                        nc.tensor.matmul(
                            pt[:],
                            WOs[:, g, dt_ * 128:(dt_ + 1) * 128],
                            rhs[:, g, th * 512:(th + 1) * 512],
                            start=(g == 0), stop=(g == 1),
                        )
                    stb = opool.tile([128, 512], BF16, tag="stB",
                                     name=f"stb_{nm}_{dt_}_{th}")
                    pcopy(stb[:], pt[:])
                    (nc.sync if th == 0 else nc.scalar).dma_start(
                        dst[dt_, :, th * 512:(th + 1) * 512], stb[:])
                return emit

            def make_agroup(nUA, dt_, rt):
                half = slice(rt * 512, (rt + 1) * 512)

                def emit():
                    pt = ps.tile([128, 512], F32, tag="mm",
                                 name=f"apt{rt}_{dt_}")
                    for g in range(2):
                        nc.tensor.matmul(
                            pt[:],
                            WOs[:, g, dt_ * 128:(dt_ + 1) * 128],
                            nUA[:, g, half],
                            start=(g == 0), stop=(g == 1),
                        )
                    st = opool.tile([128, 512], BF16, tag="stA",
                                    name=f"sta{rt}_{dt_}")
                    if rt == 0 and dt_ >= 5:
                        # tail-critical: halve the copy latency by running
                        # both engines on one group concurrently
                        nc.vector.tensor_copy(out=st[:, 0:256],
                                              in_=pt[:, 0:256])
                        nc.scalar.activation(st[:, 256:512],
                                             pt[:, 256:512], IDENT)
                    else:
                        pcopy(st[:], pt[:])
                    (nc.sync if dt_ % 2 == 0 else nc.scalar).dma_start(
                        AT[dt_, :, half], st[:])
                return emit

            # (cost, emit) units: cost ~ PE slots of ~0.45us each
            extra_work = [(1, make_bgroup(BT, gU, dt_, th, "b"))
                          for dt_ in range(8) for th in range(2)]
            if with_c:
                extra_work += [(1, make_bgroup(CT, bU, dt_, th, "c"))
                               for dt_ in range(8) for th in range(2)]

            def pop_work(budget=2, keep=0):
                # keep: hold back units so the PE stays warm during the
                # final dependency chain (STT->SILU->AV->invd->nUA->A)
                while extra_work and budget > 0 and len(extra_work) > keep:
                    cost, fn = extra_work.pop(0)
                    fn()
                    budget -= cost

            # ---- attention: row-tiled QK pairs, col-tiled AV pairs ----
            AVs = big.tile([128, 2, 1024], F32R)  # [c%128, c//128, t]
            nUA = scratch.tile([128, 2, 1024], BF16, tag="nua")
            sstage = scratch.tile([1, 4, 512], F32, tag="sst")

            def make_stats(rt, si, sqh):
                half = slice(rt * 512, (rt + 1) * 512)

                def emit():
                    sp = ps.tile([128, 512], F32, tag="mm",
                                 name=f"sp{rt}_{si}")
                    for g in range(2):
                        rhs = (AVs[:, g, half] if si == 0 else sqh[:, g, :])
                        nc.tensor.matmul(
                            sp[0:1, :], ONESs[:], rhs,
                            start=(g == 0), stop=(g == 1),
                        )
                    nc.vector.tensor_copy(
                        out=sstage[:, si * 2 + rt, :], in_=sp[0:1, :])
                return emit

            # rt1 first so the tail dependency chain is rt0's (shorter,
            # causal-trimmed) one; kt batched by 2 so partial-tile LDWEIGHTS
            # alternate row/col groups and hide behind the paired matmul
            for rt in (1, 0):
                n_kt = 4 * rt + 4
                half = slice(rt * 512, (rt + 1) * 512)
                sqh = scratch.tile([128, 2, 512], F32R, tag=f"sqh{rt}",
                                   name=f"sqh{rt}")
                for jp in (0, 2):
                    ch = jp // 2
                    avp = psav.tile([128, 512], F32, tag="av",
                                    name=f"avp{rt}_{jp}")
                    pend = []

                    def flush_one(avp=avp, ch=ch, n_kt=n_kt):
                        kt_, attb_, off_ = pend.pop(0)
                        v_emit(kt_)   # V4[:, kt_] writer must be emitted
                        # col-tiled concurrent AV pair: head jp -> psum
                        # partitions 0-63, head jp+1 -> 64-127, same bank
                        for hi in range(2):
                            nc.tensor.matmul(
                                avp[hi * 64:(hi + 1) * 64, off_:512],
                                V4[:, kt_, jp + hi, :],
                                attb_[:, hi, off_:512],
                                start=(kt_ == 0), stop=(kt_ == n_kt - 1),
                                skip_group_check=True,
                            )

                    for kt2 in range(0, n_kt, 2):
                        kts = [kt2, kt2 + 1]
                        geo = []
                        for kt in kts:
                            d0 = rt * 512 - kt * 128
                            off = max(0, -d0)   # causal-trim: i >= kt*128
                            geo.append((kt, off, max(0, d0)))
                        # 4 adjacent QK matmuls, alternating row groups:
                        # each partial LDWEIGHTS hides behind the other
                        # row group's in-flight matmul
                        qkps = []
                        for kt, off, cs in geo:
                            qkp = psqk.tile([128, 2, 512], F32, tag="qk",
                                            name=f"qkp{rt}_{jp}_{kt}")
                            qkps.append(qkp)
                            for hi in range(2):
                                pb = hi * 64
                                nc.tensor.matmul(
                                    qkp[:, hi, off:512],
                                    qk[pb:pb + 64, 2 + ch,
                                       kt * 128:(kt + 1) * 128],
                                    qk[pb:pb + 64, ch,
                                       rt * 512 + off:(rt + 1) * 512],
                                    start=True, stop=True,
                                )
                        for (kt, off, cs), qkp in zip(geo, qkps):
                            W = 512 - off
                            attb = apool.tile([128, 2, 512], BF16,
                                              tag="attb")
                            if rt == 1 and kt <= (2 if jp else 1):
                                # all distances >= 128 here: the T5 bucket
                                # saturates, rab is a per-head constant ->
                                # silu(alpha*logits + c_h) straight from
                                # psum, no mask tensor pass on DVE
                                for hi in range(2):
                                    nc.scalar.activation(
                                        attb[:, hi, :], qkp[:, hi, :],
                                        SILU,
                                        bias=PKs[:, PK_RABF + jp + hi:
                                                 PK_RABF + jp + hi + 1],
                                        scale=alpha)
                            else:
                                # pair-fused alpha*logits + mask -> bf16
                                att = apool.tile([128, 2, 512], BF16,
                                                 tag="att")
                                nc.vector.scalar_tensor_tensor(
                                    att[:, :, off:512], qkp[:, :, off:512],
                                    alpha, MTs[:, jp:jp + 2, cs:cs + W],
                                    MULT, ADD)
                                nc.scalar.activation(attb[:, :, off:512],
                                                     att[:, :, off:512],
                                                     SILU)
                            pend.append((kt, attb, off))
                        # pull V units forward as fill while V remains
                        if v_next[0] < 8:
                            v_emit(min(v_next[0], 7))
                        else:
                            pop_work(4, keep=4 if rt else 0)
                        while len(pend) > 1:
                            flush_one()
                    while pend:
                        flush_one()
                        pop_work(1, keep=4 if rt else 0)
                    # one invd multiply for the whole pair (full partitions)
                    nc.vector.tensor_mul(
                        AVs[:, ch, half], avp[:],
                        PKi[:, rt * 512:(rt + 1) * 512])
                    # per-pair squared/gated products start immediately
                    nc.gpsimd.tensor_mul(sqh[:, ch, :], AVs[:, ch, half],
                                         AVs[:, ch, half])
                    nc.gpsimd.tensor_mul(nUA[:, ch, half], AVs[:, ch, half],
                                         gU[:, ch, half])

                extra_work.extend(
                    (1, make_agroup(nUA, dt_, rt)) for dt_ in range(8))
                extra_work.extend([(1, make_stats(rt, 0, None)),
                                   (1, make_stats(rt, 1, sqh))])

            # warm dummy matmuls bridge the final dependency chain
            # (invd -> nUA -> A) so the drain runs at full clock
            for i in range(12):
                wps2 = ps.tile([128, 512], F32, tag="mm", name=f"wend{i}")
                nc.tensor.matmul(wps2[:], wrm[:, 0:128], wrm[:],
                                 start=True, stop=True,
                                 skip_group_check=True)
            while extra_work:
                extra_work.pop(0)[1]()

            nc.sync.dma_start(SOUT[:], sstage[:])

    nc.compile()
    return nc


_NC_CACHE = {}


def _prep_in_maps(inputs):
    x = np.asarray(inputs["x"], np.float32)
    key_padding_mask = np.asarray(inputs["key_padding_mask"])
    W_in = np.asarray(inputs["W_in"], np.float32)
    b_in = np.asarray(inputs["b_in"], np.float32)
    W_out = np.asarray(inputs["W_out"], np.float32)
    gamma = np.asarray(inputs["gamma"], np.float32)
    beta = np.asarray(inputs["beta"], np.float32)
    rab_emb = np.asarray(inputs["rab_emb"], np.float32)

    lengths = (~key_padding_mask).sum(axis=1)  # valid keys per batch
    in_maps = []
    for core in range(8):
        b, hg = core // 4, core % 4
        sl = slice(hg * 256, hg * 256 + 256)
        Wu = W_in[0:1024][sl]
        Wv = W_in[1024:2048][sl]
        Wq = W_in[2048:3072][sl]
        Wk = W_in[3072:4096][sl]
        WC_np = np.concatenate([Wu, Wq, Wk], 0).T.reshape(8, 128, 768)
        WVC_np = Wv.T.reshape(2, 4, 128, 256).transpose(0, 2, 1, 3)
        XC_np = x[b].T.reshape(8, 128, 1024)
        WO_np = np.ascontiguousarray(
            W_out[:, sl].T.reshape(2, 128, 1024).transpose(1, 0, 2))
        L = int(lengths[b])
        denom = np.clip(np.minimum(np.arange(T) + 1, L), 1, None)
        heads = [4 * hg + jj for jj in range(H_PER)]
        MT_np = _toeplitz_mask_np(rab_emb, heads)

        PKA_np = np.zeros((128, PKA_N), np.float32)
        bqk = np.concatenate([b_in[2048:3072][sl], b_in[3072:4096][sl]])
        PKA_np[:, PK_BQK:PK_BQK + 4] = bqk.reshape(4, 128).T
        PKA_np[:, PK_BU:PK_BU + 2] = b_in[0:1024][sl].reshape(2, 128).T
        PKA_np[:, PK_GAM:PK_GAM + 2] = gamma[sl].reshape(2, 128).T
        PKA_np[:, PK_BET:PK_BET + 2] = beta[sl].reshape(2, 128).T
        PKA_np[:, PK_VM:PK_VM + 8] = (
            np.arange(128)[:, None] + 128 * np.arange(8)[None, :] < L)
        PKA_np[:, PK_BVB:PK_BVB + 256] = b_in[1024:2048][sl][None, :]
        PKA_np[:, PK_RABF:PK_RABF + 4] = rab_emb[NUM_BUCKETS - 1, heads][None, :]
        PKB_np = np.ascontiguousarray(
            np.broadcast_to((1.0 / denom)[None, :].astype(np.float32),
                            (128, 1024)))

        in_maps.append({
            "XC": np.ascontiguousarray(XC_np).astype(BF16_NP),
            "WC": np.ascontiguousarray(WC_np).astype(BF16_NP),
            "WVC": np.ascontiguousarray(WVC_np).astype(BF16_NP),
            "WO": WO_np.astype(BF16_NP),
            "MT": MT_np.astype(BF16_NP),
            "PKA": PKA_np,
            "PKB": PKB_np,
            "ONESP": np.ones((128, 1), np.float32),
        })
    return in_maps


def kernel(x, attention_mask, key_padding_mask, W_in, b_in, W_out, b_out,
           gamma, beta, rab_emb):
    global LAST_RESULTS
    x = np.asarray(x, np.float32)
    key_padding_mask = np.asarray(key_padding_mask)
    b_out = np.asarray(b_out, np.float32)
    beta = np.asarray(beta, np.float32)

    with_c = bool(np.any(beta != 0.0))
    if with_c not in _NC_CACHE:
        _NC_CACHE[with_c] = _build(with_c)
    nc = _NC_CACHE[with_c]

    in_maps = _prep_in_maps(dict(
        x=x, attention_mask=attention_mask, key_padding_mask=key_padding_mask,
        W_in=W_in, b_in=b_in, W_out=W_out, b_out=b_out, gamma=gamma,
        beta=beta, rab_emb=rab_emb))

    res = run_bass_kernel_spmd(nc, in_maps, list(range(8)))
    LAST_RESULTS = res

    out = np.empty((B, T, D), np.float32)
    for b in range(B):
        A = np.zeros((T, D), np.float64)
        Bm = np.zeros((T, D), np.float64)
        Cm = np.zeros((T, D), np.float64)
        s1 = np.zeros(T, np.float64)
        s2 = np.zeros(T, np.float64)
        for hg in range(4):
            r = res.results[4 * b + hg]
            A += r["AT"].reshape(1024, 1024).T.astype(np.float64)
            Bm += r["BT"].reshape(1024, 1024).T.astype(np.float64)
            if with_c:
                Cm += r["CT"].reshape(1024, 1024).T.astype(np.float64)
            s = r["SOUT"].reshape(4, 512)
            s1 += np.concatenate([s[0], s[1]]).astype(np.float64)
            s2 += np.concatenate([s[2], s[3]]).astype(np.float64)
        # s1, s2 already invd-scaled on device
        mu = s1 / D
        var = s2 / D - mu * mu
        rho = 1.0 / np.sqrt(var + LN_EPS)
        y = (rho[:, None] * A - (rho * mu)[:, None] * Bm + Cm
             + b_out[None, :].astype(np.float64) + x[b].astype(np.float64))
        out[b] = y.astype(np.float32)
    return out
